# revision 16
# baseline (speedup 1.0000x reference)
"""Multi-head attention (Vaswani) on Trainium2, head-parallel across 8 NeuronCores.

Problem shapes (hardcoded):
  h:   [B=2, G=2048, D=128] f32
  W_Q/W_K/W_V: [H=8, D=128, K=16] f32
  out: [B=2, H=8, G=2048, V=16] f32  = softmax(0.25 * (h@Wq) @ (h@Wk)^T) @ (h@Wv)

Sharding: one head per core (8 heads / 8 cores). Each core receives the full h
plus its head's weight slices, computes [B, G, V]; host stacks on the head axis.

Per-core plan, all in transposed "compatT" orientation so the attention @ V
contraction lands on the partition axis with no transposes of the big G x G
attention matrix:
  1. hT[d, g] via PE transposes of [128,128] h tiles (f32r: 1.5 cyc/row).
  2. qT[17, g], kT[17, g] = Wq^T @ hT, Wk^T @ hT (f32r, 1 cyc/row). W_K is
     host-prescaled by 0.25*log2(e)*2^23 and row 16 of qT/kT is a constant
     bias pair (1.0, -0.5*2^23) so the compat psum lands directly in the
     "magic rounding" domain P = (t - 0.5)*2^23, t = log2(attention score).
  3. v'[m, 17] chunks = (h_chunk @ Wv | ones column); the ones column makes the
     softmax denominator accumulate in output row 16 for free.
  4. Per key chunk m (128 keys) and q-slice (1024 wide): compatT[m, q] into
     psum, then exp via one of two paths (static split, cfg dve_frac):
       ACT:  exp2 on the Scalar engine: activation(Exp, scale=ln2*2^-23,
             bias=0.5*ln2)  [exact]
       DVE+GPS: custom 8-op DVE instruction EXPM_ANT computes the Schraudolph
             float E = (126 + floor(t) + quad(frac))*2^23; a gpsimd
             tensor_scalar adds the mantissa constant and converts to int32
             on write -- the int bits ARE the fp32 exp value (~2.5e-3 max rel).
     oT[17, q] += v'^T @ attnT accumulated in psum over all 16 key chunks.
     Input staging for the next batch is interleaved into this loop.
  5. Transpose oT back in [17,128] blocks, scale rows by the reciprocal of
     the denominator row, one DMA per q-slice out.

The big matmul streams run as float32r (single-pass PE). End-to-end rel err
~2e-4 with dve_frac=0, ~1e-3 at dve_frac=0.35 (gate is 2e-2).
"""

import numpy as np

B, G, D = 2, 2048, 128
H, K, V = 8, 16, 16
N_CORES = 8
P = 128
GT = G // P          # 16 key/query chunks of 128
QB = 512             # one fp32 PSUM bank of free dim
NQB = G // QB        # 4
VP1 = V + 1          # v' width (ones column appended)
KP1 = K + 1          # q/k rows + bias row

LOG2E = 1.4426950408889634
LN2 = 0.6931471805599453
S_K = 0.25 * LOG2E * (2.0 ** 23)      # host-side W_K prescale
BIAS_ROW = -0.5 * 2.0 ** 23           # k-side bias row (q-side row is 1.0)
ACT_SCALE = LN2 * 2.0 ** -23
ACT_BIAS = 0.5 * LN2
# EXPM_ANT constants: quadratic mantissa fit  c + b*g + a*g^2 ~= 2^(g+1/2)
EA, EB, EC = 0.344000773, 0.995042388, 1.413999808
MAGIC = 1.5 * 2 ** 46
C1V = 1.5 * 2 ** 46 - 126 * 2 ** 23

DEFAULT_CFG = {
    "chunk_w": 1024,   # max compat psum tile width
    "pc_bufs": 2,      # compat psum buffers
    "at_bufs": 6,      # attnT sbuf buffers
    "e_bufs": 3,       # EXPM intermediate sbuf buffers
    "fp32r": True,     # float32r tiles for the big matmul streams
    "dve_frac": 0.35,  # fraction of exp tiles routed DVE+gpsimd
    "gps_copies": True,  # vp ones + oT_sb copies on gpsimd
    "reps": 1,         # repeat whole kernel body (for HW slope timing)
}

_CACHE = {}


def _register_expm():
    """Register the EXPM_ANT custom DVE op (runtime append to dve_ops.OPS).

    E = (126 + floor(t) + a*g^2 + b*g) * 2^23,  g = frac(t) - 0.5, from
    Src0 = (t - 0.5)*2^23.  Final exp(t*ln2) bits = int32(E + c*2^23).
    """
    from concourse import dve_ops
    from concourse.dve_spec import (Spec, Src0, C0 as sC0, C1 as sC1,
                                    C2 as sC2, C3 as sC3,
                                    _spill_c3_to_src1, lower)
    from concourse.dve_uop import DveOpSpec

    if "EXPM_ANT" in dve_ops._SUB_OPCODE_FOR_NAME:
        for op in dve_ops.OPS:
            if op.name == "EXPM_ANT":
                return op

    def _ref(in0, in1, s0, s1, imm2):
        f32 = np.float32
        src0 = in0.astype(f32)
        z = f32(src0 + f32(s0))
        it = f32(z - f32(s0))
        f = f32(src0 - it)
        ic = f32(z - f32(s1))
        n1 = f32(f * f32(imm2))
        n2 = f32(n1 + in1.astype(f32))
        n3 = f32(n2 * f)
        return f32(ic + n3)

    z = Src0 + sC0
    it = z - sC0
    f = Src0 - it
    ic = z - sC1
    n1 = f * sC2
    n2 = n1 + sC3
    n3 = n2 * f
    u = ic + n3
    spec = Spec(body=_spill_c3_to_src1(u), reference=_ref)

    row = max(dve_ops._SUB_OPCODE_FOR_NAME.values()) + 1
    dve_ops._SUB_OPCODE_FOR_NAME["EXPM_ANT"] = row
    uops = lower(spec, ver="v3")
    sha = DveOpSpec(name="EXPM_ANT", opcode=row, uops=uops,
                    rd1_en=True).sha("v3")
    op = dve_ops.DveOp("EXPM_ANT", spec, subdim=False, uops_sha={"v3": sha})
    dve_ops.OPS.append(op)
    dve_ops.CUSTOM_DVE_SPECS["EXPM_ANT"] = spec
    return op


def _build(cfg_key):
    cfg = dict(DEFAULT_CFG)
    cfg.update(dict(cfg_key))
    import concourse.bacc as bacc
    import concourse.mybir as mybir
    from concourse.tile import TileContext
    from concourse.masks import make_identity

    f32 = mybir.dt.float32
    f32r = mybir.dt.float32r
    i16 = mybir.dt.int16
    bf16 = mybir.dt.bfloat16
    mdt = f32r if cfg["fp32r"] else f32
    EXP = mybir.ActivationFunctionType.Exp
    ADD = mybir.AluOpType.add

    expm = _register_expm() if cfg["dve_frac"] > 0 else None

    nc = bacc.Bacc("TRN2", debug=False, enable_asserts=False,
                   target_bir_lowering=False)
    h_d = nc.dram_tensor("h", [B, G, D], f32, kind="ExternalInput").ap()
    wq_d = nc.dram_tensor("wq", [D, K], f32, kind="ExternalInput").ap()
    wk_d = nc.dram_tensor("wk", [D, K], f32, kind="ExternalInput").ap()
    wv_d = nc.dram_tensor("wv", [D, V], f32, kind="ExternalInput").ap()
    brows_d = nc.dram_tensor("brows", [2, G], f32,
                             kind="ExternalInput").ap()
    out_d = nc.dram_tensor("out", [B, G, V], f32, kind="ExternalOutput").ap()

    CW = cfg["chunk_w"]
    NCW = G // CW        # q-slices per key chunk
    n_units = 2 * GT * NCW  # exp tiles per rep (both batches)

    # static ACT/DVE schedule for exp tiles, spread evenly
    dve_sched = []
    acc = 0.0
    for _ in range(n_units):
        acc += cfg["dve_frac"]
        if acc >= 1.0 - 1e-9:
            acc -= 1.0
            dve_sched.append(True)
        else:
            dve_sched.append(False)

    with TileContext(nc) as tc:
        with tc.tile_pool(name="const", bufs=1) as cpool, \
             tc.tile_pool(name="sc", bufs=2, space="PSUM") as scpool, \
             tc.tile_pool(name="pc", bufs=cfg["pc_bufs"],
                          space="PSUM") as pcpool, \
             tc.tile_pool(name="po", bufs=1, space="PSUM") as popool, \
             tc.tile_pool(name="exp", bufs=cfg["e_bufs"]) as epool, \
             tc.tile_pool(name="att", bufs=cfg["at_bufs"]) as apool:
            ident = cpool.tile([P, P], f32)
            make_identity(nc, ident)
            warm = cpool.tile([P, 1], f32)
            nc.scalar.activation(warm, ident[:, 0:1], EXP)
            bconst = cpool.tile([P, 1], f32)
            nc.vector.memset(bconst, EB)
            actbias = cpool.tile([P, 1], f32)
            nc.vector.memset(actbias, ACT_BIAS)
            w_sb = cpool.tile([D, 3 * K], f32)
            w_r = cpool.tile([D, 3 * K], mdt)

            def load_w():
                nc.sync.dma_start(w_sb[:, 0:K], wq_d)
                nc.sync.dma_start(w_sb[:, K:2 * K], wk_d)
                nc.sync.dma_start(w_sb[:, 2 * K:3 * K], wv_d)
                nc.vector.tensor_copy(w_r, w_sb)

            one = cpool.tile([P, 1], f32)
            nc.vector.memset(one, 1.0)
            hA_b, hT_b, qkT_b, kTp_b, vp_b, ob_b = [], [], [], [], [], []
            for b in range(B):
                hA_b.append(cpool.tile([P, G], f32, name=f"hA{b}"))
                hT_b.append(cpool.tile([P, G], mdt, name=f"hT{b}"))
                qkT_b.append(cpool.tile([KP1, G], mdt, name=f"qkT{b}"))
                kTp_b.append(cpool.tile([KP1, G], mdt, name=f"kTp{b}"))
                vp_b.append(cpool.tile([P, GT * VP1], bf16, name=f"vp{b}"))
                ob_b.append(cpool.tile([P, GT * V], f32, name=f"ob{b}"))

            def init_rows():
                # constant rows, set once: q-side 1.0, k-side bias (via DMA
                # -- engines cannot start at partition 16), v' ones
                ceng = nc.gpsimd if cfg["gps_copies"] else nc.vector
                for b in range(B):
                    nc.sync.dma_start(qkT_b[b][K:KP1, :],
                                      brows_d[0:1, :].bitcast(mdt))
                    nc.sync.dma_start(kTp_b[b][K:KP1, :],
                                      brows_d[1:2, :].bitcast(mdt))
                    for t in range(GT):
                        ceng.tensor_copy(
                            vp_b[b][:, t * VP1 + V:(t + 1) * VP1], one)

            def phase1_ops(b):
                """Closure list for batch b's input staging, in dependency
                order; popped a few at a time inside the previous batch's
                main loop so the work fills engine gaps."""
                hA, hT, qkT = hA_b[b], hT_b[b], qkT_b[b]
                kTp, vp = kTp_b[b], vp_b[b]

                def dmaq(qq):
                    nc.sync.dma_start(
                        hA[:, qq * 4 * P:(qq + 1) * 4 * P].rearrange(
                            "p (t d) -> p t d", t=4),
                        h_d[b, qq * 4 * P:(qq + 1) * 4 * P, :].rearrange(
                            "(t p) d -> p t d", p=P))

                def tr(t):
                    pt = scpool.tile([P, QB], f32, tag="s", name="pt")
                    nc.tensor.transpose(pt[:, 0:P],
                                        hA[:, t * P:(t + 1) * P], ident)
                    nc.vector.tensor_copy(hT[:, t * P:(t + 1) * P],
                                          pt[:, 0:P])

                def proj(qb, w0, dst):
                    sl = slice(qb * QB, (qb + 1) * QB)
                    pq = scpool.tile([P, QB], f32, tag="s", name="pq")
                    nc.tensor.matmul(pq[0:K, :], w_r[:, w0:w0 + K],
                                     hT[:, sl], start=True, stop=True)
                    nc.vector.tensor_copy(dst[0:K, sl], pq[0:K, :])

                def vproj(t):
                    pvv = scpool.tile([P, QB], f32, tag="s", name="pvv")
                    nc.tensor.matmul(pvv[:, 0:V], hT[:, t * P:(t + 1) * P],
                                     w_r[:, 2 * K:3 * K],
                                     start=True, stop=True)
                    nc.vector.tensor_copy(vp[:, t * VP1:t * VP1 + V],
                                          pvv[:, 0:V])

                ops = [lambda: dmaq(0), lambda: dmaq(1),
                       lambda: dmaq(2), lambda: dmaq(3)]
                for qq in range(NQB):
                    for t in range(4 * qq, 4 * qq + 4):
                        ops.append(lambda t=t: tr(t))
                    ops.append(lambda qq=qq: proj(qq, 0, qkT))
                    ops.append(lambda qq=qq: proj(qq, K, kTp))
                    for t in range(4 * qq, 4 * qq + 4):
                        ops.append(lambda t=t: vproj(t))
                return ops

            units = [(rr, bb) for rr in range(cfg["reps"])
                     for bb in range(B)]
            first = phase1_ops(units[0][1])
            first = (first[0:2] + [load_w] + first[2:4] + [init_rows]
                     + first[4:])
            # prefix must cover every projection the first q-slice reads
            # (emission order IS dependency order for Tile)
            npre = 27
            for op in first[:npre]:
                op()
            pending = first[npre:]
            ui_exp = 0
            for ui, (rep, b) in enumerate(units):
                qkT, kTp, vp, ob_all = (qkT_b[b], kTp_b[b], vp_b[b],
                                        ob_b[b])
                if ui + 1 < len(units):
                    pending = pending + phase1_ops(units[ui + 1][1])

                sched = [(ci * CW, CW) for ci in range(NCW)]
                for si, (q0, width) in enumerate(sched):
                    oT = popool.tile([VP1, CW], f32, tag="oT",
                                     name="oT")[:, 0:width]
                    for t in range(GT):
                        v_sl = vp[:, t * VP1:(t + 1) * VP1]
                        cps = pcpool.tile([P, CW], f32, tag="c",
                                          name="cps")[:, 0:width]
                        kT_sl = kTp[0:KP1, t * P:(t + 1) * P]
                        for j in range(width // QB):
                            nc.tensor.matmul(
                                cps[:, j * QB:(j + 1) * QB], kT_sl,
                                qkT[0:KP1,
                                    q0 + j * QB:q0 + (j + 1) * QB],
                                start=True, stop=True)
                        at = apool.tile([P, CW], bf16, tag="at",
                                        name="at")[:, 0:width]
                        use_dve = dve_sched[(ui_exp + t) % n_units]
                        if expm is not None and use_dve:
                            et = epool.tile([P, CW], f32, tag="e",
                                            name="et")[:, 0:width]
                            nc.vector._custom_dve(
                                expm, out=et, in0=cps, in1=bconst,
                                s0=MAGIC, s1=C1V, imm2=EA / 2 ** 23)
                            # bf16 bits = int16(E*2^-16 + c*2^7)
                            nc.gpsimd.tensor_scalar(
                                at.bitcast(i16), et, 2.0 ** -16,
                                EC * 2 ** 7, mybir.AluOpType.mult, ADD)
                        else:
                            nc.scalar.activation(at, cps, EXP,
                                                 scale=ACT_SCALE,
                                                 bias=actbias)
                        for j in range(width // QB):
                            nc.tensor.matmul(
                                oT[:, j * QB:(j + 1) * QB], v_sl,
                                at[:, j * QB:(j + 1) * QB],
                                start=(t == 0), stop=(t == GT - 1))
                        # emit a few staged ops for the NEXT batch; end-of-
                        # chunk placement keeps them behind this chunk's
                        # matmuls in the PE queue while still preceding
                        # every consumer
                        for _ in range(3):
                            if pending:
                                pending.pop(0)()
                    ui_exp += GT

                    # normalize this q-slice
                    oT_sb = apool.tile([VP1, CW], f32, tag="oTsb",
                                       name="oT_sb")[:, 0:width]
                    NT = width // P
                    half = width // 2
                    # gpsimd cannot read PSUM -- these stay on DVE
                    nc.vector.tensor_copy(oT_sb[:, 0:half], oT[:, 0:half])
                    nc.vector.tensor_copy(oT_sb[:, half:width],
                                          oT[:, half:width])
                    for tl in range(NT):
                        tg = (q0 + tl * P) // P
                        pf = scpool.tile([P, QB], f32, tag="s",
                                         name="pf")
                        nc.tensor.transpose(
                            pf[:, 0:VP1],
                            oT_sb[:, tl * P:(tl + 1) * P],
                            ident[:VP1, :VP1])
                        rcp = apool.tile([P, 1], f32, tag="rcp",
                                         name="rcp")
                        nc.vector.reciprocal(rcp, pf[:, V:V + 1])
                        nc.vector.tensor_scalar_mul(
                            ob_all[:, tg * V:(tg + 1) * V],
                            pf[:, 0:V], rcp)

                    # per-slice out DMA so the store overlaps the next
                    nc.sync.dma_start(
                        out_d[b, q0:q0 + width, :].rearrange(
                            "(t p) v -> p t v", p=P),
                        ob_all[:, (q0 // P) * V:((q0 + width) // P) * V]
                        .rearrange("p (t v) -> p t v", t=width // P))

                for op in pending:
                    op()
                pending = []

    nc.compile()
    return nc


def _get(cfg=None):
    cfg = cfg or {}
    key = tuple(sorted({**DEFAULT_CFG, **cfg}.items()))
    if key not in _CACHE:
        _CACHE[key] = _build(key)
    return _CACHE[key]


def _in_maps(h, W_Q, W_K, W_V):
    h = np.ascontiguousarray(np.asarray(h, dtype=np.float32))
    W_Q = np.asarray(W_Q, dtype=np.float32)
    W_K = np.asarray(W_K, dtype=np.float32) * np.float32(S_K)
    W_V = np.asarray(W_V, dtype=np.float32)
    brows = np.empty((2, G), dtype=np.float32)
    brows[0] = 1.0
    brows[1] = BIAS_ROW
    return [
        {"h": h, "wq": np.ascontiguousarray(W_Q[c]),
         "wk": np.ascontiguousarray(W_K[c]),
         "wv": np.ascontiguousarray(W_V[c]),
         "brows": brows}
        for c in range(N_CORES)
    ]


def kernel(h, W_Q, W_K, W_V, cfg=None, **run_kwargs):
    from concourse import bass_utils
    nc = _get(cfg)
    res = bass_utils.run_bass_kernel_spmd(
        nc, _in_maps(h, W_Q, W_K, W_V),
        core_ids=list(range(N_CORES)), **run_kwargs)
    out = np.stack([res.results[c]["out"] for c in range(N_CORES)], axis=1)
    kernel.last_results = res
    return out


# revision 24
# speedup vs baseline: 1.0370x; 1.0370x over previous
"""Multi-head attention (Vaswani) on Trainium2, head-parallel across 8 NeuronCores.

Problem shapes (hardcoded):
  h:   [B=2, G=2048, D=128] f32
  W_Q/W_K/W_V: [H=8, D=128, K=16] f32
  out: [B=2, H=8, G=2048, V=16] f32  = softmax(0.25 * (h@Wq) @ (h@Wk)^T) @ (h@Wv)

Sharding: one head per core (8 heads / 8 cores). Each core receives the full h
plus its head's weight slices, computes [B, G, V]; host stacks on the head axis.

Per-core plan, all in transposed "compatT" orientation so the attention @ V
contraction lands on the partition axis with no transposes of the big G x G
attention matrix:
  1. hT[d, g] via PE transposes of [128,128] h tiles (f32r: 1.5 cyc/row).
  2. qT[17, g], kT[17, g] = Wq^T @ hT, Wk^T @ hT (f32r, 1 cyc/row). W_K is
     host-prescaled by 0.25*log2(e)*2^23 and row 16 of qT/kT is a constant
     bias pair (1.0, -0.5*2^23) so the compat psum lands directly in the
     "magic rounding" domain P = (t - 0.5)*2^23, t = log2(attention score).
  3. v'[m, 17] chunks = (h_chunk @ Wv | ones column); the ones column makes the
     softmax denominator accumulate in output row 16 for free.
  4. Per key chunk m (128 keys) and q-slice (1024 wide): compatT[m, q] into
     psum, then exp via one of two paths (static split, cfg dve_frac):
       ACT:  exp2 on the Scalar engine: activation(Exp, scale=ln2*2^-23,
             bias=0.5*ln2)  [exact]
       DVE+GPS: custom 8-op DVE instruction EXPM_ANT computes the Schraudolph
             float E = (126 + floor(t) + quad(frac))*2^23; a gpsimd
             tensor_scalar adds the mantissa constant and converts to int32
             on write -- the int bits ARE the fp32 exp value (~2.5e-3 max rel).
     oT[17, q] += v'^T @ attnT accumulated in psum over all 16 key chunks.
     Input staging for the next batch is interleaved into this loop.
  5. Transpose oT back in [17,128] blocks, scale rows by the reciprocal of
     the denominator row, one DMA per q-slice out.

The big matmul streams run as float32r (single-pass PE). End-to-end rel err
~2e-4 with dve_frac=0, ~1e-3 at dve_frac=0.35 (gate is 2e-2).
"""

import numpy as np

B, G, D = 2, 2048, 128
H, K, V = 8, 16, 16
N_CORES = 8
P = 128
GT = G // P          # 16 key/query chunks of 128
QB = 512             # one fp32 PSUM bank of free dim
NQB = G // QB        # 4
VP1 = V + 1          # v' width (ones column appended)
KP1 = K + 1          # q/k rows + bias row

LOG2E = 1.4426950408889634
LN2 = 0.6931471805599453
S_K = 0.25 * LOG2E * (2.0 ** 23)      # host-side W_K prescale
BIAS_ROW = -0.5 * 2.0 ** 23           # k-side bias row (q-side row is 1.0)
ACT_SCALE = LN2 * 2.0 ** -23
ACT_BIAS = 0.5 * LN2
# EXPM_ANT constants: quadratic mantissa fit  c + b*g + a*g^2 ~= 2^(g+1/2)
EA, EB, EC = 0.344000773, 0.995042388, 1.413999808
MAGIC = 1.5 * 2 ** 46
C1V = 1.5 * 2 ** 46 - 126 * 2 ** 23

DEFAULT_CFG = {
    "chunk_w": 1024,   # max compat psum tile width
    "pc_bufs": 2,      # compat psum buffers
    "po_bufs": 1,      # oT psum buffers
    "at_bufs": 8,      # attnT sbuf buffers
    "e_bufs": 3,       # EXPM intermediate sbuf buffers
    "fp32r": True,     # float32r tiles for the big matmul streams
    "dve_frac": 0.35,  # per-chunk fraction of exp columns on DVE+gpsimd
    "fin_gps": 1.0,    # fraction of the DVE part finished on gpsimd
    "gps_copies": True,  # vp ones copies on gpsimd
    "reps": 1,         # repeat whole kernel body (for HW slope timing)
}

_CACHE = {}


def _register_expm():
    """Register the EXPM_ANT custom DVE op (runtime append to dve_ops.OPS).

    E = (126 + floor(t) + a*g^2 + b*g) * 2^23,  g = frac(t) - 0.5, from
    Src0 = (t - 0.5)*2^23.  Final exp(t*ln2) bits = int32(E + c*2^23).
    """
    from concourse import dve_ops
    from concourse.dve_spec import (Spec, Src0, C0 as sC0, C1 as sC1,
                                    C2 as sC2, C3 as sC3,
                                    _spill_c3_to_src1, lower)
    from concourse.dve_uop import DveOpSpec

    if "EXPM_ANT" in dve_ops._SUB_OPCODE_FOR_NAME:
        for op in dve_ops.OPS:
            if op.name == "EXPM_ANT":
                return op

    def _ref(in0, in1, s0, s1, imm2):
        f32 = np.float32
        src0 = in0.astype(f32)
        z = f32(src0 + f32(s0))
        it = f32(z - f32(s0))
        f = f32(src0 - it)
        ic = f32(z - f32(s1))
        n1 = f32(f * f32(imm2))
        n2 = f32(n1 + in1.astype(f32))
        n3 = f32(n2 * f)
        return f32(ic + n3)

    z = Src0 + sC0
    it = z - sC0
    f = Src0 - it
    ic = z - sC1
    n1 = f * sC2
    n2 = n1 + sC3
    n3 = n2 * f
    u = ic + n3
    spec = Spec(body=_spill_c3_to_src1(u), reference=_ref)

    row = max(dve_ops._SUB_OPCODE_FOR_NAME.values()) + 1
    dve_ops._SUB_OPCODE_FOR_NAME["EXPM_ANT"] = row
    uops = lower(spec, ver="v3")
    sha = DveOpSpec(name="EXPM_ANT", opcode=row, uops=uops,
                    rd1_en=True).sha("v3")
    op = dve_ops.DveOp("EXPM_ANT", spec, subdim=False, uops_sha={"v3": sha})
    dve_ops.OPS.append(op)
    dve_ops.CUSTOM_DVE_SPECS["EXPM_ANT"] = spec
    return op


def _build(cfg_key):
    cfg = dict(DEFAULT_CFG)
    cfg.update(dict(cfg_key))
    import concourse.bacc as bacc
    import concourse.mybir as mybir
    from concourse.tile import TileContext
    from concourse.masks import make_identity

    f32 = mybir.dt.float32
    f32r = mybir.dt.float32r
    i16 = mybir.dt.int16
    bf16 = mybir.dt.bfloat16
    mdt = f32r if cfg["fp32r"] else f32
    EXP = mybir.ActivationFunctionType.Exp
    ADD = mybir.AluOpType.add

    expm = _register_expm() if cfg["dve_frac"] > 0 else None

    nc = bacc.Bacc("TRN2", debug=False, enable_asserts=False,
                   target_bir_lowering=False)
    h_d = nc.dram_tensor("h", [B, G, D], f32, kind="ExternalInput").ap()
    wq_d = nc.dram_tensor("wq", [D, K], f32, kind="ExternalInput").ap()
    wk_d = nc.dram_tensor("wk", [D, K], f32, kind="ExternalInput").ap()
    wv_d = nc.dram_tensor("wv", [D, V], f32, kind="ExternalInput").ap()
    brows_d = nc.dram_tensor("brows", [2, G], f32,
                             kind="ExternalInput").ap()
    out_d = nc.dram_tensor("out", [B, G, V], f32, kind="ExternalOutput").ap()

    CW = cfg["chunk_w"]
    NCW = G // CW        # q-slices per key chunk

    # per-chunk exp column split: [0:wa] ACT, [wa:] DVE custom op, whose
    # output is finished (bf16-bits convert) on gpsimd [wa:wa+wg] / DVE rest
    wd = int(round(CW * cfg["dve_frac"] / 8)) * 8
    wa = CW - wd
    wg = int(round(wd * cfg["fin_gps"] / 8)) * 8

    with TileContext(nc) as tc:
        with tc.tile_pool(name="const", bufs=1) as cpool, \
             tc.tile_pool(name="sc", bufs=2, space="PSUM") as scpool, \
             tc.tile_pool(name="pc", bufs=cfg["pc_bufs"],
                          space="PSUM") as pcpool, \
             tc.tile_pool(name="po", bufs=cfg["po_bufs"],
                          space="PSUM") as popool, \
             tc.tile_pool(name="exp", bufs=cfg["e_bufs"]) as epool, \
             tc.tile_pool(name="att", bufs=cfg["at_bufs"]) as apool:
            ident = cpool.tile([P, P], f32)
            make_identity(nc, ident)
            warm = cpool.tile([P, 1], f32)
            nc.scalar.activation(warm, ident[:, 0:1], EXP)
            bconst = cpool.tile([P, 1], f32)
            nc.vector.memset(bconst, EB)
            actbias = cpool.tile([P, 1], f32)
            nc.vector.memset(actbias, ACT_BIAS)
            w_sb = cpool.tile([D, 3 * K], f32)
            w_r = cpool.tile([D, 3 * K], mdt)

            def load_w():
                nc.sync.dma_start(w_sb[:, 0:K], wq_d)
                nc.sync.dma_start(w_sb[:, K:2 * K], wk_d)
                nc.sync.dma_start(w_sb[:, 2 * K:3 * K], wv_d)
                nc.vector.tensor_copy(w_r, w_sb)

            one = cpool.tile([P, 1], f32)
            nc.vector.memset(one, 1.0)
            hA_b, hT_b, qkT_b, kTp_b, vp_b, ob_b = [], [], [], [], [], []
            for b in range(B):
                hA_b.append(cpool.tile([P, G], f32, name=f"hA{b}"))
                hT_b.append(cpool.tile([P, G], mdt, name=f"hT{b}"))
                qkT_b.append(cpool.tile([KP1, G], mdt, name=f"qkT{b}"))
                kTp_b.append(cpool.tile([KP1, G], mdt, name=f"kTp{b}"))
                vp_b.append(cpool.tile([P, GT * VP1], bf16, name=f"vp{b}"))
                ob_b.append(cpool.tile([P, GT * V], f32, name=f"ob{b}"))

            def init_rows():
                # constant rows, set once: q-side 1.0, k-side bias (via DMA
                # -- engines cannot start at partition 16), v' ones
                ceng = nc.gpsimd if cfg["gps_copies"] else nc.vector
                for b in range(B):
                    nc.sync.dma_start(qkT_b[b][K:KP1, :],
                                      brows_d[0:1, :].bitcast(mdt))
                    nc.sync.dma_start(kTp_b[b][K:KP1, :],
                                      brows_d[1:2, :].bitcast(mdt))
                    for t in range(GT):
                        ceng.tensor_copy(
                            vp_b[b][:, t * VP1 + V:(t + 1) * VP1], one)

            def phase1_ops(b):
                """Closure list for batch b's input staging, in dependency
                order; popped a few at a time inside the previous batch's
                main loop so the work fills engine gaps."""
                hA, hT, qkT = hA_b[b], hT_b[b], qkT_b[b]
                kTp, vp = kTp_b[b], vp_b[b]

                def dmaq(qq):
                    nc.sync.dma_start(
                        hA[:, qq * 4 * P:(qq + 1) * 4 * P].rearrange(
                            "p (t d) -> p t d", t=4),
                        h_d[b, qq * 4 * P:(qq + 1) * 4 * P, :].rearrange(
                            "(t p) d -> p t d", p=P))

                def tr(t):
                    pt = scpool.tile([P, QB], f32, tag="s", name="pt")
                    nc.tensor.transpose(pt[:, 0:P],
                                        hA[:, t * P:(t + 1) * P], ident)
                    nc.vector.tensor_copy(hT[:, t * P:(t + 1) * P],
                                          pt[:, 0:P])

                def proj(qb, w0, dst):
                    sl = slice(qb * QB, (qb + 1) * QB)
                    pq = scpool.tile([P, QB], f32, tag="s", name="pq")
                    nc.tensor.matmul(pq[0:K, :], w_r[:, w0:w0 + K],
                                     hT[:, sl], start=True, stop=True)
                    nc.vector.tensor_copy(dst[0:K, sl], pq[0:K, :])

                def vproj(t):
                    pvv = scpool.tile([P, QB], f32, tag="s", name="pvv")
                    nc.tensor.matmul(pvv[:, 0:V], hT[:, t * P:(t + 1) * P],
                                     w_r[:, 2 * K:3 * K],
                                     start=True, stop=True)
                    nc.vector.tensor_copy(vp[:, t * VP1:t * VP1 + V],
                                          pvv[:, 0:V])

                ops = [lambda: dmaq(0), lambda: dmaq(1),
                       lambda: dmaq(2), lambda: dmaq(3)]
                for qq in range(NQB):
                    for t in range(4 * qq, 4 * qq + 4):
                        ops.append(lambda t=t: tr(t))
                    ops.append(lambda qq=qq: proj(qq, 0, qkT))
                    ops.append(lambda qq=qq: proj(qq, K, kTp))
                    for t in range(4 * qq, 4 * qq + 4):
                        ops.append(lambda t=t: vproj(t))
                return ops

            units = [(rr, bb) for rr in range(cfg["reps"])
                     for bb in range(B)]
            first = phase1_ops(units[0][1])
            first = (first[0:2] + [load_w] + first[2:4] + [init_rows]
                     + first[4:])
            # prefix must cover every projection the first q-slice reads
            # (emission order IS dependency order for Tile)
            npre = 27
            for op in first[:npre]:
                op()
            pending = first[npre:]
            for ui, (rep, b) in enumerate(units):
                qkT, kTp, vp, ob_all = (qkT_b[b], kTp_b[b], vp_b[b],
                                        ob_b[b])
                if ui + 1 < len(units):
                    pending = pending + phase1_ops(units[ui + 1][1])

                sched = [(ci * CW, CW) for ci in range(NCW)]
                for si, (q0, width) in enumerate(sched):
                    oT = popool.tile([VP1, CW], f32, tag="oT",
                                     name="oT")[:, 0:width]
                    for t in range(GT):
                        v_sl = vp[:, t * VP1:(t + 1) * VP1]
                        cps = pcpool.tile([P, CW], f32, tag="c",
                                          name="cps")[:, 0:width]
                        kT_sl = kTp[0:KP1, t * P:(t + 1) * P]
                        for j in range(width // QB):
                            nc.tensor.matmul(
                                cps[:, j * QB:(j + 1) * QB], kT_sl,
                                qkT[0:KP1,
                                    q0 + j * QB:q0 + (j + 1) * QB],
                                start=True, stop=True)
                        at = apool.tile([P, CW], bf16, tag="at",
                                        name="at")[:, 0:width]
                        # DVE part at columns [0:wd] (offset-0 PSUM read),
                        # ACT part at [wd:width]
                        if wd < width:
                            nc.scalar.activation(at[:, wd:width],
                                                 cps[:, wd:width], EXP,
                                                 scale=ACT_SCALE,
                                                 bias=actbias)
                        if expm is not None and wd > 0:
                            et = epool.tile([P, CW], f32, tag="e",
                                            name="et")
                            nc.vector._custom_dve(
                                expm, out=et[:, 0:wd],
                                in0=cps[:, 0:wd], in1=bconst,
                                s0=MAGIC, s1=C1V, imm2=EA / 2 ** 23)
                            # bf16 bits = int16(E*2^-16 + c*2^7)
                            if wg > 0:
                                nc.gpsimd.tensor_scalar(
                                    at.bitcast(i16)[:, 0:wg],
                                    et[:, 0:wg], 2.0 ** -16,
                                    EC * 2 ** 7, mybir.AluOpType.mult,
                                    ADD)
                            if wg < wd:
                                nc.vector.tensor_scalar(
                                    at.bitcast(i16)[:, wg:wd],
                                    et[:, wg:wd], 2.0 ** -16,
                                    EC * 2 ** 7, mybir.AluOpType.mult,
                                    ADD)
                        for j in range(width // QB):
                            nc.tensor.matmul(
                                oT[:, j * QB:(j + 1) * QB], v_sl,
                                at[:, j * QB:(j + 1) * QB],
                                start=(t == 0), stop=(t == GT - 1))
                        # emit a few staged ops for the NEXT batch; end-of-
                        # chunk placement keeps them behind this chunk's
                        # matmuls in the PE queue while still preceding
                        # every consumer
                        for _ in range(3):
                            if pending:
                                pending.pop(0)()

                    # normalize this q-slice
                    oT_sb = apool.tile([VP1, CW], f32, tag="oTsb",
                                       name="oT_sb")[:, 0:width]
                    NT = width // P
                    # gpsimd cannot read PSUM -- this stays on DVE
                    nc.vector.tensor_copy(oT_sb, oT)
                    for tl in range(NT):
                        tg = (q0 + tl * P) // P
                        pf = scpool.tile([P, QB], f32, tag="s",
                                         name="pf")
                        nc.tensor.transpose(
                            pf[:, 0:VP1],
                            oT_sb[:, tl * P:(tl + 1) * P],
                            ident[:VP1, :VP1])
                        rcp = apool.tile([P, 1], f32, tag="rcp",
                                         name="rcp")
                        nc.vector.reciprocal(rcp, pf[:, V:V + 1])
                        nc.vector.tensor_scalar_mul(
                            ob_all[:, tg * V:(tg + 1) * V],
                            pf[:, 0:V], rcp)

                    # per-slice out DMA so the store overlaps the next
                    nc.sync.dma_start(
                        out_d[b, q0:q0 + width, :].rearrange(
                            "(t p) v -> p t v", p=P),
                        ob_all[:, (q0 // P) * V:((q0 + width) // P) * V]
                        .rearrange("p (t v) -> p t v", t=width // P))

                for op in pending:
                    op()
                pending = []

    nc.compile()
    return nc


def _get(cfg=None):
    cfg = cfg or {}
    key = tuple(sorted({**DEFAULT_CFG, **cfg}.items()))
    if key not in _CACHE:
        _CACHE[key] = _build(key)
    return _CACHE[key]


def _in_maps(h, W_Q, W_K, W_V):
    h = np.ascontiguousarray(np.asarray(h, dtype=np.float32))
    W_Q = np.asarray(W_Q, dtype=np.float32)
    W_K = np.asarray(W_K, dtype=np.float32) * np.float32(S_K)
    W_V = np.asarray(W_V, dtype=np.float32)
    brows = np.empty((2, G), dtype=np.float32)
    brows[0] = 1.0
    brows[1] = BIAS_ROW
    return [
        {"h": h, "wq": np.ascontiguousarray(W_Q[c]),
         "wk": np.ascontiguousarray(W_K[c]),
         "wv": np.ascontiguousarray(W_V[c]),
         "brows": brows}
        for c in range(N_CORES)
    ]


def kernel(h, W_Q, W_K, W_V, cfg=None, **run_kwargs):
    from concourse import bass_utils
    nc = _get(cfg)
    res = bass_utils.run_bass_kernel_spmd(
        nc, _in_maps(h, W_Q, W_K, W_V),
        core_ids=list(range(N_CORES)), **run_kwargs)
    out = np.stack([res.results[c]["out"] for c in range(N_CORES)], axis=1)
    kernel.last_results = res
    return out


# revision 28
# speedup vs baseline: 1.1083x; 1.0687x over previous
"""Multi-head attention (Vaswani) on Trainium2, head-parallel across 8 NeuronCores.

Problem shapes (hardcoded):
  h:   [B=2, G=2048, D=128] f32
  W_Q/W_K/W_V: [H=8, D=128, K=16] f32
  out: [B=2, H=8, G=2048, V=16] f32  = softmax(0.25 * (h@Wq) @ (h@Wk)^T) @ (h@Wv)

Sharding: one head per core (8 heads / 8 cores). Each core receives the full h
plus its head's weight slices, computes [B, G, V]; host stacks on the head axis.

Per-core plan, all in transposed "compatT" orientation so the attention @ V
contraction lands on the partition axis with no transposes of the big G x G
attention matrix:
  1. hT[d, g] via PE transposes of [128,128] h tiles (f32r: 1.5 cyc/row).
  2. qT[17, g], kT[17, g] = Wq^T @ hT, Wk^T @ hT (f32r, 1 cyc/row). W_K is
     host-prescaled by 0.25*log2(e)*2^23 and row 16 of qT/kT is a constant
     bias pair (1.0, -0.5*2^23) so the compat psum lands directly in the
     "magic rounding" domain P = (t - 0.5)*2^23, t = log2(attention score).
  3. v'[m, 17] chunks = (h_chunk @ Wv | ones column); the ones column makes the
     softmax denominator accumulate in output row 16 for free.
  4. Per key chunk m (128 keys) and q-slice (1024 wide): compatT[m, q] into
     psum, then exp into a bf16 attnT tile via a per-chunk column split:
       ACT  cols [wd:1024]: activation(Exp, scale=ln2*2^-23, bias=0.5*ln2)
             [exact, bf16 rounding only]
       DVE+GPS cols [0:wd]: custom 8-op DVE instruction EXPM_ANT computes the
             Schraudolph float E = (126 + floor(t) + quad(frac))*2^23; a
             gpsimd (and/or DVE) tensor_scalar computes int16(E*2^-16 + c*2^7)
             -- those int bits ARE the bf16 exp value (~0.5% max rel).
     oT[17, q] += v'^T @ attnT accumulated in psum over all 16 key chunks
     (bf16 matmul, 1 cyc/row). Input staging for the next batch is
     interleaved into this loop.
  5. Transpose oT back in [17,128] blocks, scale rows by the reciprocal of
     the denominator row, one DMA per q-slice out.

The big matmul streams run as float32r (single-pass PE). End-to-end rel err
~2e-4 with dve_frac=0, ~1e-3 at dve_frac=0.35 (gate is 2e-2).
"""

import numpy as np

B, G, D = 2, 2048, 128
H, K, V = 8, 16, 16
N_CORES = 8
P = 128
GT = G // P          # 16 key/query chunks of 128
QB = 512             # one fp32 PSUM bank of free dim
NQB = G // QB        # 4
VP1 = V + 1          # v' width (ones column appended)
KP1 = K + 1          # q/k rows + bias row

LOG2E = 1.4426950408889634
LN2 = 0.6931471805599453
S_K = 0.25 * LOG2E * (2.0 ** 23)      # host-side W_K prescale
BIAS_ROW = -0.5 * 2.0 ** 23           # k-side bias row (q-side row is 1.0)
ACT_SCALE = LN2 * 2.0 ** -23
ACT_BIAS = 0.5 * LN2
# EXPM_ANT constants: quadratic mantissa fit  c + b*g + a*g^2 ~= 2^(g+1/2)
EA, EB, EC = 0.344000773, 0.995042388, 1.413999808
MAGIC = 1.5 * 2 ** 46
C1V = 1.5 * 2 ** 46 - 126 * 2 ** 23

DEFAULT_CFG = {
    "chunk_w": 1024,   # max compat psum tile width
    "pc_bufs": 2,      # compat psum buffers
    "po_bufs": 1,      # oT psum buffers
    "at_bufs": 8,      # attnT sbuf buffers
    "e_bufs": 3,       # EXPM intermediate sbuf buffers
    "fp32r": True,     # float32r tiles for the big matmul streams
    "dve_frac": 0.35,  # per-chunk fraction of exp columns on DVE+gpsimd
    "fin_gps": 1.0,    # fraction of the DVE part finished on gpsimd
    "gps_copies": True,  # vp ones copies on gpsimd
    "av_delay": 1,     # chunks of AV lag behind QK (software pipeline)
    "reps": 1,         # repeat whole kernel body (for HW slope timing)
}

_CACHE = {}


def _register_expm():
    """Register the EXPM_ANT custom DVE op (runtime append to dve_ops.OPS).

    E = (126 + floor(t) + a*g^2 + b*g) * 2^23,  g = frac(t) - 0.5, from
    Src0 = (t - 0.5)*2^23.  Final exp(t*ln2) bits = int32(E + c*2^23).
    """
    from concourse import dve_ops
    from concourse.dve_spec import (Spec, Src0, C0 as sC0, C1 as sC1,
                                    C2 as sC2, C3 as sC3,
                                    _spill_c3_to_src1, lower)
    from concourse.dve_uop import DveOpSpec

    if "EXPM_ANT" in dve_ops._SUB_OPCODE_FOR_NAME:
        for op in dve_ops.OPS:
            if op.name == "EXPM_ANT":
                return op

    def _ref(in0, in1, s0, s1, imm2):
        f32 = np.float32
        src0 = in0.astype(f32)
        z = f32(src0 + f32(s0))
        it = f32(z - f32(s0))
        f = f32(src0 - it)
        ic = f32(z - f32(s1))
        n1 = f32(f * f32(imm2))
        n2 = f32(n1 + in1.astype(f32))
        n3 = f32(n2 * f)
        return f32(ic + n3)

    z = Src0 + sC0
    it = z - sC0
    f = Src0 - it
    ic = z - sC1
    n1 = f * sC2
    n2 = n1 + sC3
    n3 = n2 * f
    u = ic + n3
    spec = Spec(body=_spill_c3_to_src1(u), reference=_ref)

    row = max(dve_ops._SUB_OPCODE_FOR_NAME.values()) + 1
    dve_ops._SUB_OPCODE_FOR_NAME["EXPM_ANT"] = row
    uops = lower(spec, ver="v3")
    sha = DveOpSpec(name="EXPM_ANT", opcode=row, uops=uops,
                    rd1_en=True).sha("v3")
    op = dve_ops.DveOp("EXPM_ANT", spec, subdim=False, uops_sha={"v3": sha})
    dve_ops.OPS.append(op)
    dve_ops.CUSTOM_DVE_SPECS["EXPM_ANT"] = spec
    return op


def _build(cfg_key):
    cfg = dict(DEFAULT_CFG)
    cfg.update(dict(cfg_key))
    import concourse.bacc as bacc
    import concourse.mybir as mybir
    from concourse.tile import TileContext
    from concourse.masks import make_identity

    f32 = mybir.dt.float32
    f32r = mybir.dt.float32r
    i16 = mybir.dt.int16
    bf16 = mybir.dt.bfloat16
    mdt = f32r if cfg["fp32r"] else f32
    EXP = mybir.ActivationFunctionType.Exp
    ADD = mybir.AluOpType.add

    expm = _register_expm() if cfg["dve_frac"] > 0 else None

    nc = bacc.Bacc("TRN2", debug=False, enable_asserts=False,
                   target_bir_lowering=False)
    h_d = nc.dram_tensor("h", [B, G, D], f32, kind="ExternalInput").ap()
    wq_d = nc.dram_tensor("wq", [D, K], f32, kind="ExternalInput").ap()
    wk_d = nc.dram_tensor("wk", [D, K], f32, kind="ExternalInput").ap()
    wv_d = nc.dram_tensor("wv", [D, V], f32, kind="ExternalInput").ap()
    brows_d = nc.dram_tensor("brows", [2, G], f32,
                             kind="ExternalInput").ap()
    out_d = nc.dram_tensor("out", [B, G, V], f32, kind="ExternalOutput").ap()

    CW = cfg["chunk_w"]
    NCW = G // CW        # q-slices per key chunk

    # per-chunk exp column split: [0:wd] DVE custom op (finished on gpsimd
    # [0:wg] / DVE [wg:wd]), [wd:width] ACT
    wd = int(round(CW * cfg["dve_frac"] / 8)) * 8
    wg = int(round(wd * cfg["fin_gps"] / 8)) * 8

    with TileContext(nc) as tc:
        with tc.tile_pool(name="const", bufs=1) as cpool, \
             tc.tile_pool(name="sc", bufs=2, space="PSUM") as scpool, \
             tc.tile_pool(name="pc", bufs=cfg["pc_bufs"],
                          space="PSUM") as pcpool, \
             tc.tile_pool(name="po", bufs=cfg["po_bufs"],
                          space="PSUM") as popool, \
             tc.tile_pool(name="exp", bufs=cfg["e_bufs"]) as epool, \
             tc.tile_pool(name="att", bufs=cfg["at_bufs"]) as apool:
            ident = cpool.tile([P, P], f32)
            make_identity(nc, ident)
            warm = cpool.tile([P, 1], f32)
            nc.scalar.activation(warm, ident[:, 0:1], EXP)
            bconst = cpool.tile([P, 1], f32)
            nc.vector.memset(bconst, EB)
            actbias = cpool.tile([P, 1], f32)
            nc.vector.memset(actbias, ACT_BIAS)
            w_sb = cpool.tile([D, 3 * K], f32)
            w_r = cpool.tile([D, 3 * K], mdt)

            def load_w():
                nc.sync.dma_start(w_sb[:, 0:K], wq_d)
                nc.sync.dma_start(w_sb[:, K:2 * K], wk_d)
                nc.sync.dma_start(w_sb[:, 2 * K:3 * K], wv_d)
                nc.vector.tensor_copy(w_r, w_sb)

            one = cpool.tile([P, 1], f32)
            nc.vector.memset(one, 1.0)
            hA_b, hT_b, qkT_b, kTp_b, vp_b, ob_b = [], [], [], [], [], []
            for b in range(B):
                hA_b.append(cpool.tile([P, G], f32, name=f"hA{b}"))
                hT_b.append(cpool.tile([P, G], mdt, name=f"hT{b}"))
                qkT_b.append(cpool.tile([KP1, G], mdt, name=f"qkT{b}"))
                kTp_b.append(cpool.tile([KP1, G], mdt, name=f"kTp{b}"))
                vp_b.append(cpool.tile([P, GT * VP1], bf16, name=f"vp{b}"))
                ob_b.append(cpool.tile([P, GT * V], f32, name=f"ob{b}"))

            def init_rows():
                # constant rows, set once: q-side 1.0, k-side bias (via DMA
                # -- engines cannot start at partition 16), v' ones
                ceng = nc.gpsimd if cfg["gps_copies"] else nc.vector
                for b in range(B):
                    nc.sync.dma_start(qkT_b[b][K:KP1, :],
                                      brows_d[0:1, :].bitcast(mdt))
                    nc.sync.dma_start(kTp_b[b][K:KP1, :],
                                      brows_d[1:2, :].bitcast(mdt))
                    for t in range(GT):
                        ceng.tensor_copy(
                            vp_b[b][:, t * VP1 + V:(t + 1) * VP1], one)

            def phase1_ops(b):
                """Closure list for batch b's input staging, in dependency
                order; popped a few at a time inside the previous batch's
                main loop so the work fills engine gaps."""
                hA, hT, qkT = hA_b[b], hT_b[b], qkT_b[b]
                kTp, vp = kTp_b[b], vp_b[b]

                def dmaq(qq):
                    nc.sync.dma_start(
                        hA[:, qq * 4 * P:(qq + 1) * 4 * P].rearrange(
                            "p (t d) -> p t d", t=4),
                        h_d[b, qq * 4 * P:(qq + 1) * 4 * P, :].rearrange(
                            "(t p) d -> p t d", p=P))

                def tr(t):
                    pt = scpool.tile([P, QB], f32, tag="s", name="pt")
                    nc.tensor.transpose(pt[:, 0:P],
                                        hA[:, t * P:(t + 1) * P], ident)
                    nc.vector.tensor_copy(hT[:, t * P:(t + 1) * P],
                                          pt[:, 0:P])

                def proj(qb, w0, dst):
                    sl = slice(qb * QB, (qb + 1) * QB)
                    pq = scpool.tile([P, QB], f32, tag="s", name="pq")
                    nc.tensor.matmul(pq[0:K, :], w_r[:, w0:w0 + K],
                                     hT[:, sl], start=True, stop=True)
                    nc.vector.tensor_copy(dst[0:K, sl], pq[0:K, :])

                def vproj(t):
                    pvv = scpool.tile([P, QB], f32, tag="s", name="pvv")
                    nc.tensor.matmul(pvv[:, 0:V], hT[:, t * P:(t + 1) * P],
                                     w_r[:, 2 * K:3 * K],
                                     start=True, stop=True)
                    nc.vector.tensor_copy(vp[:, t * VP1:t * VP1 + V],
                                          pvv[:, 0:V])

                ops = [lambda: dmaq(0), lambda: dmaq(1),
                       lambda: dmaq(2), lambda: dmaq(3)]
                for qq in range(NQB):
                    for t in range(4 * qq, 4 * qq + 4):
                        ops.append(lambda t=t: tr(t))
                    ops.append(lambda qq=qq: proj(qq, 0, qkT))
                    ops.append(lambda qq=qq: proj(qq, K, kTp))
                    for t in range(4 * qq, 4 * qq + 4):
                        ops.append(lambda t=t: vproj(t))
                return ops

            units = [(rr, bb) for rr in range(cfg["reps"])
                     for bb in range(B)]
            first = phase1_ops(units[0][1])
            first = (first[0:2] + [load_w] + first[2:4] + [init_rows]
                     + first[4:])
            # prefix must cover every projection the first q-slice reads
            # (emission order IS dependency order for Tile)
            npre = 27
            for op in first[:npre]:
                op()
            pending = first[npre:]
            for ui, (rep, b) in enumerate(units):
                qkT, kTp, vp, ob_all = (qkT_b[b], kTp_b[b], vp_b[b],
                                        ob_b[b])
                if ui + 1 < len(units):
                    pending = pending + phase1_ops(units[ui + 1][1])

                sched = [(ci * CW, CW) for ci in range(NCW)]
                for si, (q0, width) in enumerate(sched):
                    oT = popool.tile([VP1, CW], f32, tag="oT",
                                     name="oT")[:, 0:width]

                    def emit_av(tt, at_t):
                        v_sl = vp[:, tt * VP1:(tt + 1) * VP1]
                        for j in range(width // QB):
                            nc.tensor.matmul(
                                oT[:, j * QB:(j + 1) * QB], v_sl,
                                at_t[:, j * QB:(j + 1) * QB],
                                start=(tt == 0), stop=(tt == GT - 1))

                    # software pipeline: AV(t) is emitted av_delay chunks
                    # behind QK(t) so the in-order PE queue never parks on
                    # AV waiting for exp -- the next chunk's QK (and staged
                    # phase-1 PE work) runs while exp(t) is in flight.
                    av_q = []
                    for t in range(GT):
                        cps = pcpool.tile([P, CW], f32, tag="c",
                                          name="cps")[:, 0:width]
                        kT_sl = kTp[0:KP1, t * P:(t + 1) * P]
                        for j in range(width // QB):
                            nc.tensor.matmul(
                                cps[:, j * QB:(j + 1) * QB], kT_sl,
                                qkT[0:KP1,
                                    q0 + j * QB:q0 + (j + 1) * QB],
                                start=True, stop=True)
                        at = apool.tile([P, CW], bf16, tag="at",
                                        name="at")[:, 0:width]
                        # DVE part at columns [0:wd] (offset-0 PSUM read),
                        # ACT part at [wd:width]
                        if wd < width:
                            nc.scalar.activation(at[:, wd:width],
                                                 cps[:, wd:width], EXP,
                                                 scale=ACT_SCALE,
                                                 bias=actbias)
                        if expm is not None and wd > 0:
                            et = epool.tile([P, CW], f32, tag="e",
                                            name="et")
                            nc.vector._custom_dve(
                                expm, out=et[:, 0:wd],
                                in0=cps[:, 0:wd], in1=bconst,
                                s0=MAGIC, s1=C1V, imm2=EA / 2 ** 23)
                            # bf16 bits = int16(E*2^-16 + c*2^7)
                            if wg > 0:
                                nc.gpsimd.tensor_scalar(
                                    at.bitcast(i16)[:, 0:wg],
                                    et[:, 0:wg], 2.0 ** -16,
                                    EC * 2 ** 7, mybir.AluOpType.mult,
                                    ADD)
                            if wg < wd:
                                nc.vector.tensor_scalar(
                                    at.bitcast(i16)[:, wg:wd],
                                    et[:, wg:wd], 2.0 ** -16,
                                    EC * 2 ** 7, mybir.AluOpType.mult,
                                    ADD)
                        av_q.append((t, at))
                        if len(av_q) > cfg["av_delay"]:
                            emit_av(*av_q.pop(0))
                        # emit a few staged ops for the NEXT batch; end-of-
                        # chunk placement keeps them behind this chunk's
                        # matmuls in the PE queue while still preceding
                        # every consumer
                        for _ in range(3):
                            if pending:
                                pending.pop(0)()
                    for tt, at_t in av_q:
                        emit_av(tt, at_t)

                    # normalize this q-slice
                    oT_sb = apool.tile([VP1, CW], f32, tag="oTsb",
                                       name="oT_sb")[:, 0:width]
                    NT = width // P
                    # gpsimd cannot read PSUM -- this stays on DVE
                    nc.vector.tensor_copy(oT_sb, oT)
                    for tl in range(NT):
                        tg = (q0 + tl * P) // P
                        pf = scpool.tile([P, QB], f32, tag="s",
                                         name="pf")
                        nc.tensor.transpose(
                            pf[:, 0:VP1],
                            oT_sb[:, tl * P:(tl + 1) * P],
                            ident[:VP1, :VP1])
                        rcp = apool.tile([P, 1], f32, tag="rcp",
                                         name="rcp")
                        nc.vector.reciprocal(rcp, pf[:, V:V + 1])
                        nc.vector.tensor_scalar_mul(
                            ob_all[:, tg * V:(tg + 1) * V],
                            pf[:, 0:V], rcp)

                    # per-slice out DMA so the store overlaps the next
                    nc.sync.dma_start(
                        out_d[b, q0:q0 + width, :].rearrange(
                            "(t p) v -> p t v", p=P),
                        ob_all[:, (q0 // P) * V:((q0 + width) // P) * V]
                        .rearrange("p (t v) -> p t v", t=width // P))

                for op in pending:
                    op()
                pending = []

    nc.compile()
    return nc


def _get(cfg=None):
    cfg = cfg or {}
    key = tuple(sorted({**DEFAULT_CFG, **cfg}.items()))
    if key not in _CACHE:
        _CACHE[key] = _build(key)
    return _CACHE[key]


def _in_maps(h, W_Q, W_K, W_V):
    h = np.ascontiguousarray(np.asarray(h, dtype=np.float32))
    W_Q = np.asarray(W_Q, dtype=np.float32)
    W_K = np.asarray(W_K, dtype=np.float32) * np.float32(S_K)
    W_V = np.asarray(W_V, dtype=np.float32)
    brows = np.empty((2, G), dtype=np.float32)
    brows[0] = 1.0
    brows[1] = BIAS_ROW
    return [
        {"h": h, "wq": np.ascontiguousarray(W_Q[c]),
         "wk": np.ascontiguousarray(W_K[c]),
         "wv": np.ascontiguousarray(W_V[c]),
         "brows": brows}
        for c in range(N_CORES)
    ]


def kernel(h, W_Q, W_K, W_V, cfg=None, **run_kwargs):
    from concourse import bass_utils
    nc = _get(cfg)
    res = bass_utils.run_bass_kernel_spmd(
        nc, _in_maps(h, W_Q, W_K, W_V),
        core_ids=list(range(N_CORES)), **run_kwargs)
    out = np.stack([res.results[c]["out"] for c in range(N_CORES)], axis=1)
    kernel.last_results = res
    return out
